# revision 6
# baseline (speedup 1.0000x reference)
"""Bass/Trainium2 kernel for per-site routed dense layer (MoE-style routing).

Computation (reference):
    Wg = weight[channels]            # (L, Cout, Cin)
    y  = einsum('blc,loc->blo', x, Wg) + bias[channels]
    y  = tanh(y) + x
    returns (y, channels)

Strategy:
  - Data-parallel over B across 8 cores (8 batches per core); weight/bias
    tables replicated per core (MoE expert-dispatch style).
  - Host-side dispatch: sort sites by channel (argsort of `channels`), and
    hand each core x pre-transposed to (CIN, L_sorted * B_core) so the device
    streams dense per-channel segment GEMMs with the channel's weight
    stationary on the PE array:  yt = tanh(W_n @ xt + b_n) + xt.
  - All cores share one program (segment boundaries depend only on
    `channels`, which is shared), so a single SPMD Bass program runs on all
    8 cores with per-core xt data.
  - Device loop streams 2 MiB column chunks: DMA-in -> matmul (per channel
    segment, <=512 moving cols fp32) -> ACT tanh(+per-partition bias) ->
    DVE residual add -> DMA-out.  Memory-bound by design (~64 MiB HBM
    traffic per core).
"""

import numpy as np

B, L, CIN, COUT, NCH = 64, 8192, 128, 128, 64
NCORES = 8
BC = B // NCORES          # 8 batches per core
NCOLS = L * BC            # 65536 columns per core
P = 128
CHUNK = 2048              # columns per DMA chunk (1 MiB fp32)
MMAX = 512                # max fp32 moving columns per matmul / PSUM bank

_cache = {}


def _build_program(channels: np.ndarray, chunk=2048, xbufs=6, ybufs=6,
                   tbufs=4, pbufs=4, out_engine="scalar", w_engine="gpsimd",
                   in_engines=("sync",), wsplit=4):
    """Build + compile the SPMD Bass program for a given channel assignment."""
    import concourse.bacc as bacc
    import concourse.tile as tile
    import concourse.mybir as mybir

    ch = np.asarray(channels).astype(np.int64)
    counts = np.bincount(ch, minlength=NCH)
    ends = np.cumsum(counts)
    starts = ends - counts

    # Column segments (in the sorted-site * BC column space), one per
    # non-empty channel: [(col_start, col_end, n), ...] covering [0, NCOLS).
    segments = [
        (int(starts[n] * BC), int(ends[n] * BC), int(n))
        for n in range(NCH)
        if counts[n] > 0
    ]

    nc = bacc.Bacc(None, target_bir_lowering=False)
    with tile.TileContext(nc) as tc:
        with tc.tile_pool(name="dram", bufs=1, space="DRAM") as dram:
            xt = dram.tile([P, NCOLS], mybir.dt.float32, kind="ExternalInput",
                           name="xt", uniquify=False)
            wt = dram.tile([P, NCH * COUT], mybir.dt.float32, kind="ExternalInput",
                           name="wt", uniquify=False)
            bt = dram.tile([P, NCH], mybir.dt.float32, kind="ExternalInput",
                           name="bt", uniquify=False)
            yt = dram.tile([P, NCOLS], mybir.dt.float32, kind="ExternalOutput",
                           name="yt", uniquify=False)

            with tc.tile_pool(name="const", bufs=1) as cpool, \
                 tc.tile_pool(name="xin", bufs=xbufs) as xpool, \
                 tc.tile_pool(name="yout", bufs=ybufs) as ypool, \
                 tc.tile_pool(name="ytmp", bufs=tbufs) as tpool, \
                 tc.tile_pool(name="psum", bufs=pbufs, space="PSUM") as ppool:

                w_sb = cpool.tile([P, NCH * COUT], mybir.dt.float32)
                b_sb = cpool.tile([P, NCH], mybir.dt.float32)
                w_eng = getattr(nc, w_engine)
                w_eng.dma_start(out=b_sb[:], in_=bt[:])
                wpiece = NCH * COUT // wsplit
                for wi in range(wsplit):
                    w_eng.dma_start(out=w_sb[:, wi * wpiece:(wi + 1) * wpiece],
                                    in_=wt[:, wi * wpiece:(wi + 1) * wpiece])

                out_eng = getattr(nc, out_engine)
                for ci, c0 in enumerate(range(0, NCOLS, chunk)):
                    c1 = c0 + chunk
                    xin = xpool.tile([P, chunk], mybir.dt.float32)
                    in_eng = getattr(nc, in_engines[ci % len(in_engines)])
                    in_eng.dma_start(out=xin[:], in_=xt[:, c0:c1])
                    yout = ypool.tile([P, chunk], mybir.dt.float32)

                    for (s, e, n) in segments:
                        s, e = max(s, c0), min(e, c1)
                        if s >= e:
                            continue
                        for p0 in range(s, e, MMAX):
                            p1 = min(p0 + MMAX, e)
                            w = p1 - p0
                            lo = p0 - c0
                            ps = ppool.tile([P, MMAX], mybir.dt.float32)
                            nc.tensor.matmul(
                                out=ps[:, :w],
                                lhsT=w_sb[:, n * COUT:(n + 1) * COUT],
                                rhs=xin[:, lo:lo + w],
                                start=True, stop=True,
                            )
                            th = tpool.tile([P, MMAX], mybir.dt.float32)
                            nc.scalar.activation(
                                out=th[:, :w],
                                in_=ps[:, :w],
                                func=mybir.ActivationFunctionType.Tanh,
                                bias=b_sb[:, n:n + 1],
                            )
                            nc.vector.tensor_add(
                                out=yout[:, lo:lo + w],
                                in0=th[:, :w],
                                in1=xin[:, lo:lo + w],
                            )
                    out_eng.dma_start(out=yt[:, c0:c1], in_=yout[:])
    nc.compile()
    return nc


def kernel(x, channels, weight, bias):
    from concourse.bass_utils import run_bass_kernel_spmd

    x = np.asarray(x)
    ch_in = np.asarray(channels)
    weight = np.asarray(weight, dtype=np.float32)
    bias = np.asarray(bias, dtype=np.float32)

    key = ch_in.tobytes()
    if key not in _cache:
        _cache[key] = _build_program(ch_in)
    nc = _cache[key]

    perm = np.argsort(ch_in.astype(np.int64), kind="stable")

    # wt[c, n*COUT + o] = weight[n, o, c]  (lhsT slice per channel = W_n^T)
    wt = np.ascontiguousarray(weight.transpose(2, 0, 1)).reshape(CIN, NCH * COUT)
    bt = np.ascontiguousarray(bias.T)  # (COUT, NCH)

    in_maps = []
    for k in range(NCORES):
        xs_k = x[k * BC:(k + 1) * BC][:, perm]          # (BC, L, CIN) gathered
        xt_k = np.ascontiguousarray(xs_k.transpose(2, 1, 0)).reshape(P, NCOLS)
        in_maps.append({"xt": xt_k, "wt": wt, "bt": bt})

    res = run_bass_kernel_spmd(nc, in_maps, core_ids=list(range(NCORES)))

    y = np.empty((B, L, CIN), dtype=np.float32)
    for k in range(NCORES):
        yt_k = res.results[k]["yt"]
        y[k * BC:(k + 1) * BC][:, perm] = (
            yt_k.reshape(P, L, BC).transpose(2, 1, 0)
        )
    return (y, ch_in)
